# revision 1
# baseline (speedup 1.0000x reference)
"""Trainium2 Bass kernel: ContextCrossAttention (B,C,H,W)=(8,512,128,128).

Math per batch element b (algebraically collapsed from the reference):
  q      = Wq @ ctx_b + bq                          (C,)
  qks    = (q @ Wk) * C**-0.5                       (C,)     # logits = qks . x[:, hw] (+ shift, dropped)
  p[hw]  = exp(logits[hw]);  Z = sum(p)                      # softmax shift-invariance: no max-subtract
  pooled = x_b @ p                                  (C,)
  gate   = (Wv @ pooled) / Z + bv                   (C,)
  out_b  = x_b * gate[:, None]

Sharding: pure data-parallel over batch; core i handles batch element i.
"""

import numpy as np
from contextlib import ExitStack

import concourse.bass as bass
import concourse.bacc as bacc
import concourse.tile as tile
from concourse import mybir
from concourse.bass_utils import run_bass_kernel_spmd

F32 = mybir.dt.float32
F32R = mybir.dt.float32r
AF = mybir.ActivationFunctionType
OP = mybir.AluOpType

B, C, D, H, W = 8, 512, 512, 128, 128
HW = H * W                      # 16384
P = 128                         # partitions
CCH = C // P                    # 4 channel chunks
NCORES = 8
G = 8                           # hw groups
GW = HW // G                    # 2048 group width
NS = GW // 512                  # 4 matmul slices per group
SCALE = float(C) ** -0.5

XT_BUFS = 15                    # also the number of x tiles cached into pass C
PGW = 1024                      # psum logits group width (2 banks, double-buffered)
NH = GW // PGW                  # 2 psum halves per DMA group


def _build_kernel():
    nc = bacc.Bacc(
        "TRN2",
        target_bir_lowering=False,
        debug=False,
        enable_asserts=False,
        num_devices=NCORES,
    )

    xd = nc.dram_tensor("xb", [C, HW], F32, kind="ExternalInput")
    ctxd = nc.dram_tensor("ctxc", [P, CCH], F32, kind="ExternalInput")   # ctx[j*128+p] at [p, j]
    wqtd = nc.dram_tensor("wqt", [D, C], F32, kind="ExternalInput")      # Wq.T  (d, o)
    wkd = nc.dram_tensor("wk", [C, C], F32, kind="ExternalInput")        # Wk    (o, c)
    wvtd = nc.dram_tensor("wvt", [C, C], F32, kind="ExternalInput")      # Wv.T  (c, o)
    bqd = nc.dram_tensor("bqc", [P, CCH], F32, kind="ExternalInput")
    bvd = nc.dram_tensor("bvc", [P, CCH], F32, kind="ExternalInput")
    outd = nc.dram_tensor("out", [C, HW], F32, kind="ExternalOutput")

    with tile.TileContext(nc) as tc, ExitStack() as ctx:
        singles = ctx.enter_context(tc.tile_pool(name="singles", bufs=1))
        xt = ctx.enter_context(tc.tile_pool(name="xt", bufs=XT_BUFS))
        scr = ctx.enter_context(tc.tile_pool(name="scr", bufs=1))
        outp = ctx.enter_context(tc.tile_pool(name="outp", bufs=5))
        pbp = ctx.enter_context(tc.tile_pool(name="pbp", bufs=2))
        psb = ctx.enter_context(tc.tile_pool(name="psb", bufs=2))
        pslog = ctx.enter_context(tc.tile_pool(name="pslog", bufs=2, space="PSUM"))
        pssm = ctx.enter_context(tc.tile_pool(name="pssm", bufs=2, space="PSUM"))

        # ---- load weights / small inputs ----
        def _load(name, dram, shape):
            t = singles.tile(shape, F32, tag=name, name=name)
            nc.sync.dma_start(t[:], dram[:])
            return t

        wqt_sb = [None] * CCH
        wk_sb = [None] * CCH
        wvt_sb = [None] * CCH
        for j in range(CCH):
            wqt_sb[j] = singles.tile([P, C], F32, tag=f"wqt{j}", name=f"wqt{j}")
            nc.sync.dma_start(wqt_sb[j][:], wqtd[j * P:(j + 1) * P, :])
            wk_sb[j] = singles.tile([P, C], F32, tag=f"wk{j}", name=f"wk{j}")
            nc.sync.dma_start(wk_sb[j][:], wkd[j * P:(j + 1) * P, :])
            wvt_sb[j] = singles.tile([P, C], F32, tag=f"wvt{j}", name=f"wvt{j}")
            nc.sync.dma_start(wvt_sb[j][:], wvtd[j * P:(j + 1) * P, :])
        ctx_sb = _load("ctx", ctxd, [P, CCH])
        bq_sb = _load("bq", bqd, [P, CCH])
        bv_sb = _load("bv", bvd, [P, CCH])

        ones_sb = singles.tile([1, P], F32, tag="ones")
        nc.vector.memset(ones_sb[:], 1.0)

        q_sb = singles.tile([P, CCH], F32, tag="q")
        qks_sb = singles.tile([P, CCH], F32, tag="qks")
        pooled_sb = singles.tile([P, CCH], F32, tag="pooled")
        gate_sb = singles.tile([P, CCH], F32, tag="gate")
        zcols = singles.tile([1, G * NH], F32, tag="zcols")
        pcols = [singles.tile([P, G * NH], F32, tag=f"pcols{cc}", name=f"pcols{cc}") for cc in range(CCH)]
        z_sb = singles.tile([1, 1], F32, tag="z")
        rz_sb = singles.tile([P, 1], F32, tag="rz")

        # ---- q = Wq @ ctx + bq  (chunk-major [P, CCH]) ----
        for oc in range(CCH):
            pq = pssm.tile([P, 1], F32, tag="pssm", name="pssm_t")
            for dc in range(CCH):
                nc.tensor.matmul(
                    pq[:], wqt_sb[dc][:, oc * P:(oc + 1) * P], ctx_sb[:, dc:dc + 1],
                    start=(dc == 0), stop=(dc == CCH - 1),
                )
            nc.vector.tensor_add(q_sb[:, oc:oc + 1], pq[:], bq_sb[:, oc:oc + 1])

        # ---- qks = (q @ Wk) * scale ----
        for cc in range(CCH):
            pqk = pssm.tile([P, 1], F32, tag="pssm", name="pssm_t")
            for oc in range(CCH):
                nc.tensor.matmul(
                    pqk[:], wk_sb[oc][:, cc * P:(cc + 1) * P], q_sb[:, oc:oc + 1],
                    start=(oc == 0), stop=(oc == CCH - 1),
                )
            nc.scalar.mul(qks_sb[:, cc:cc + 1], pqk[:], SCALE)

        # ---- fused pass A+B: logits -> exp -> pooled partials ----
        x_tiles = {}
        for g in range(G):
            for cc in range(CCH):
                t = xt.tile([P, GW], F32, tag="x", name="x_t")
                nc.sync.dma_start(t[:], xd[cc * P:(cc + 1) * P, g * GW:(g + 1) * GW])
                x_tiles[(cc, g)] = t
            for h in range(NH):
                gh = g * NH + h
                plog = pslog.tile([1, PGW], F32, tag="plog", name="plog_t")
                for s in range(PGW // 512):
                    for cc in range(CCH):
                        nc.tensor.matmul(
                            plog[:, s * 512:(s + 1) * 512],
                            qks_sb[:, cc:cc + 1],
                            x_tiles[(cc, g)][:, h * PGW + s * 512:h * PGW + (s + 1) * 512],
                            start=(cc == 0), stop=(cc == CCH - 1),
                        )
                p_t = psb.tile([1, PGW], F32, tag="p", name="p_t")
                nc.scalar.activation(
                    p_t[:], plog[:], AF.Exp, accum_out=zcols[:, gh:gh + 1],
                )
                pb = pbp.tile([P, PGW], F32, tag="pb", name="pb_t")
                nc.gpsimd.partition_broadcast(pb[:], p_t[:])
                for cc in range(CCH):
                    sc = scr.tile([P, PGW], F32, tag="scr", name="scr_t")
                    nc.vector.scalar_tensor_tensor(
                        sc[:], x_tiles[(cc, g)][:, h * PGW:(h + 1) * PGW], 1.0, pb[:],
                        op0=OP.mult, op1=OP.mult,
                        accum_out=pcols[cc][:, gh:gh + 1],
                    )

        # ---- finalize: Z, pooled, gate = (Wv @ pooled)/Z + bv ----
        nc.vector.reduce_sum(z_sb[:], zcols[:], axis=mybir.AxisListType.X)
        zps = pssm.tile([P, 1], F32, tag="pssm", name="pssm_t")
        nc.tensor.matmul(zps[:], ones_sb[:], z_sb[:])
        nc.vector.reciprocal(rz_sb[:], zps[:])
        for cc in range(CCH):
            nc.vector.reduce_sum(
                pooled_sb[:, cc:cc + 1], pcols[cc][:], axis=mybir.AxisListType.X
            )
        for oc in range(CCH):
            pg = pssm.tile([P, 1], F32, tag="pssm", name="pssm_t")
            for cc in range(CCH):
                nc.tensor.matmul(
                    pg[:], wvt_sb[cc][:, oc * P:(oc + 1) * P], pooled_sb[:, cc:cc + 1],
                    start=(cc == 0), stop=(cc == CCH - 1),
                )
            nc.vector.scalar_tensor_tensor(
                gate_sb[:, oc:oc + 1], pg[:], rz_sb[:], bv_sb[:, oc:oc + 1],
                op0=OP.mult, op1=OP.add,
            )

        # ---- pass C: out = x * gate ----
        # last XT_BUFS x tiles of pass A+B are still resident in the xt pool:
        # multiply them in place first (no re-DMA), then stream the rest fresh.
        n_xt = G * CCH
        first_cached = n_xt - XT_BUFS

        def _pass_c(idx, t):
            g, cc = divmod(idx, CCH)
            o = outp.tile([P, GW], F32, tag="o", name="o_t")
            nc.vector.tensor_scalar_mul(o[:], t[:], gate_sb[:, cc:cc + 1])
            eng = nc.scalar if idx % 2 == 0 else nc.gpsimd
            eng.dma_start(outd[cc * P:(cc + 1) * P, g * GW:(g + 1) * GW], o[:])

        for idx in range(first_cached, n_xt):
            g, cc = divmod(idx, CCH)
            _pass_c(idx, x_tiles[(cc, g)])
        for idx in range(first_cached):
            g, cc = divmod(idx, CCH)
            t = xt.tile([P, GW], F32, tag="x", name="x_t")
            nc.sync.dma_start(t[:], xd[cc * P:(cc + 1) * P, g * GW:(g + 1) * GW])
            _pass_c(idx, t)

    nc.compile()
    return nc


_NC = None


def _get_nc():
    global _NC
    if _NC is None:
        _NC = _build_kernel()
    return _NC


def _make_in_maps(x, context, Wq, bq, Wk, bk, Wv, bv):
    x = np.ascontiguousarray(np.asarray(x, dtype=np.float32))
    wqt = np.ascontiguousarray(np.asarray(Wq, dtype=np.float32).T)
    wk = np.ascontiguousarray(np.asarray(Wk, dtype=np.float32))
    wvt = np.ascontiguousarray(np.asarray(Wv, dtype=np.float32).T)
    bqc = np.ascontiguousarray(np.asarray(bq, dtype=np.float32).reshape(CCH, P).T)
    bvc = np.ascontiguousarray(np.asarray(bv, dtype=np.float32).reshape(CCH, P).T)
    context = np.asarray(context, dtype=np.float32)
    in_maps = []
    for b in range(NCORES):
        ctxc = np.ascontiguousarray(context[b].reshape(CCH, P).T)
        in_maps.append({
            "xb": x[b].reshape(C, HW),
            "ctxc": ctxc,
            "wqt": wqt,
            "wk": wk,
            "wvt": wvt,
            "bqc": bqc,
            "bvc": bvc,
        })
    return in_maps


def run_spmd(x, context, Wq, bq, Wk, bk, Wv, bv, **spmd_kwargs):
    """Run on 8 NeuronCores; returns (output (B,C,H,W) f32, BassKernelResults)."""
    nc = _get_nc()
    in_maps = _make_in_maps(x, context, Wq, bq, Wk, bk, Wv, bv)
    res = run_bass_kernel_spmd(nc, in_maps, list(range(NCORES)), **spmd_kwargs)
    out = np.stack([
        np.asarray(res.results[b]["out"], dtype=np.float32).reshape(C, H, W)
        for b in range(NCORES)
    ])
    return out, res


def kernel(x, context, Wq, bq, Wk, bk, Wv, bv):
    out, _ = run_spmd(x, context, Wq, bq, Wk, bk, Wv, bv)
    return out



# revision 2
# speedup vs baseline: 1.1188x; 1.1188x over previous
"""Trainium2 Bass kernel: ContextCrossAttention (B,C,H,W)=(8,512,128,128).

Math per batch element b (algebraically collapsed from the reference):
  q      = Wq @ ctx_b + bq                          (C,)
  qks    = (q @ Wk) * C**-0.5                       (C,)     # logits = qks . x[:, hw] (+ shift, dropped)
  p[hw]  = exp(logits[hw]);  Z = sum(p)                      # softmax shift-invariance: no max-subtract
  pooled = x_b @ p                                  (C,)
  gate   = (Wv @ pooled) / Z + bv                   (C,)
  out_b  = x_b * gate[:, None]

Sharding: pure data-parallel over batch; core i handles batch element i.

Pipeline (two DMA-bound phases, ~64 MiB mandatory HBM traffic per core):
  read phase:  stream x (f32) -> convert to an fp16 SBUF cache (scalar/DVE),
               logits matmuls + exp + pooled accumulation all read the fp16
               cache (PE runs fp16 at full rate; f32 would be ~4x slower and
               tensor-bound).  Full 16 MiB fp16 cache fits in SBUF, so:
  write phase: out = x_fp16 * gate streamed straight from the cache -> zero
               re-read of x from HBM.
"""

import numpy as np
from contextlib import ExitStack

import concourse.bass as bass
import concourse.bacc as bacc
import concourse.tile as tile
from concourse import mybir
from concourse.bass_utils import run_bass_kernel_spmd

F32 = mybir.dt.float32
F16 = mybir.dt.float16
AF = mybir.ActivationFunctionType
OP = mybir.AluOpType

B, C, D, H, W = 8, 512, 512, 128, 128
HW = H * W                      # 16384
P = 128                         # partitions
CCH = C // P                    # 4 channel chunks
NCORES = 8
G = 8                           # hw groups
GW = HW // G                    # 2048 group width
PGW = 1024                      # psum logits tile width (2 banks, double-buffered)
NH = GW // PGW                  # 2 psum tiles per group
SCALE = float(C) ** -0.5


def _build_kernel():
    nc = bacc.Bacc(
        "TRN2",
        target_bir_lowering=False,
        debug=False,
        enable_asserts=False,
        num_devices=NCORES,
    )

    xd = nc.dram_tensor("xb", [C, HW], F32, kind="ExternalInput")
    ctxd = nc.dram_tensor("ctxc", [P, CCH], F16, kind="ExternalInput")   # ctx[j*128+p] at [p, j]
    wqtd = nc.dram_tensor("wqt", [D, C], F16, kind="ExternalInput")      # Wq.T  (d, o)
    wkd = nc.dram_tensor("wk", [C, C], F16, kind="ExternalInput")        # Wk    (o, c)
    wvtd = nc.dram_tensor("wvt", [C, C], F32, kind="ExternalInput")      # Wv.T  (c, o)
    bqd = nc.dram_tensor("bqc", [P, CCH], F32, kind="ExternalInput")
    bvd = nc.dram_tensor("bvc", [P, CCH], F32, kind="ExternalInput")
    outd = nc.dram_tensor("out", [C, HW], F32, kind="ExternalOutput")

    with tile.TileContext(nc) as tc, ExitStack() as ctx:
        singles = ctx.enter_context(tc.tile_pool(name="singles", bufs=1))
        xt = ctx.enter_context(tc.tile_pool(name="xt", bufs=3))          # f32 landing
        pbp = ctx.enter_context(tc.tile_pool(name="pbp", bufs=2))        # fp16 p broadcast
        scr = ctx.enter_context(tc.tile_pool(name="scr", bufs=1))        # fp16 STT product sink
        psb = ctx.enter_context(tc.tile_pool(name="psb", bufs=2))        # fp16 p row
        outp = ctx.enter_context(tc.tile_pool(name="outp", bufs=2))      # f32 out staging
        pslog = ctx.enter_context(tc.tile_pool(name="pslog", bufs=2, space="PSUM"))
        pssm = ctx.enter_context(tc.tile_pool(name="pssm", bufs=2, space="PSUM"))

        # ---- weights / small inputs: all on the scalar (Act) DMA ring so the
        # sync (SP) ring carries nothing but the x stream ----
        wqt_sb = [None] * CCH
        wk_sb = [None] * CCH
        wvt_sb = [None] * CCH
        ctx_sb = singles.tile([P, CCH], F16, tag="ctx", name="ctx")
        nc.scalar.dma_start(ctx_sb[:], ctxd[:])
        bq_sb = singles.tile([P, CCH], F32, tag="bq", name="bq")
        nc.scalar.dma_start(bq_sb[:], bqd[:])
        for j in range(CCH):
            wqt_sb[j] = singles.tile([P, C], F16, tag=f"wqt{j}", name=f"wqt{j}")
            nc.scalar.dma_start(wqt_sb[j][:], wqtd[j * P:(j + 1) * P, :])
            wk_sb[j] = singles.tile([P, C], F16, tag=f"wk{j}", name=f"wk{j}")
            nc.scalar.dma_start(wk_sb[j][:], wkd[j * P:(j + 1) * P, :])
        bv_sb = singles.tile([P, CCH], F32, tag="bv", name="bv")
        nc.scalar.dma_start(bv_sb[:], bvd[:])
        for j in range(CCH):
            wvt_sb[j] = singles.tile([P, C], F32, tag=f"wvt{j}", name=f"wvt{j}")
            nc.scalar.dma_start(wvt_sb[j][:], wvtd[j * P:(j + 1) * P, :])

        ones_sb = singles.tile([1, P], F32, tag="ones")
        nc.vector.memset(ones_sb[:], 1.0)

        q_sb = singles.tile([P, CCH], F16, tag="q")
        qks_sb = singles.tile([P, CCH], F16, tag="qks")
        pooled_sb = singles.tile([P, CCH], F32, tag="pooled")
        gate_sb = singles.tile([P, CCH], F32, tag="gate")
        zcols = singles.tile([1, G * NH], F32, tag="zcols")
        pcols = [singles.tile([P, G], F32, tag=f"pcols{cc}", name=f"pcols{cc}") for cc in range(CCH)]
        z_sb = singles.tile([1, 1], F32, tag="z")
        rz_sb = singles.tile([P, 1], F32, tag="rz")

        # ---- q = Wq @ ctx + bq  (chunk-major [P, CCH]) ----
        for oc in range(CCH):
            pq = pssm.tile([P, 1], F32, tag="pssm", name="pssm_t")
            for dc in range(CCH):
                nc.tensor.matmul(
                    pq[:], wqt_sb[dc][:, oc * P:(oc + 1) * P], ctx_sb[:, dc:dc + 1],
                    start=(dc == 0), stop=(dc == CCH - 1),
                )
            nc.vector.tensor_add(q_sb[:, oc:oc + 1], pq[:], bq_sb[:, oc:oc + 1])

        # ---- qks = (q @ Wk) * scale ----
        for cc in range(CCH):
            pqk = pssm.tile([P, 1], F32, tag="pssm", name="pssm_t")
            for oc in range(CCH):
                nc.tensor.matmul(
                    pqk[:], wk_sb[oc][:, cc * P:(cc + 1) * P], q_sb[:, oc:oc + 1],
                    start=(oc == 0), stop=(oc == CCH - 1),
                )
            nc.scalar.mul(qks_sb[:, cc:cc + 1], pqk[:], SCALE)

        # ---- read phase: stream x in, convert to fp16 cache, fused
        # logits -> exp -> pooled partials.  Converts for group g+1 are
        # emitted BEFORE exp/STT of group g (per-engine FIFO software
        # pipelining) so the landing pool recycles without stalling the
        # x DMA stream behind the exp/broadcast/STT chain. ----
        xcache = {}

        def emit_group_loads(g):
            for cc in range(CCH):
                t = xt.tile([P, GW], F32, tag="x", name="x_t")
                nc.sync.dma_start(t[:], xd[cc * P:(cc + 1) * P, g * GW:(g + 1) * GW])
                xc = singles.tile([P, GW], F16, tag=f"xc{g}_{cc}", name=f"xc{g}_{cc}")
                if cc % 2 == 0:
                    nc.scalar.copy(xc[:], t[:])
                else:
                    nc.vector.tensor_copy(xc[:], t[:])
                xcache[(cc, g)] = xc

        def emit_group_compute(g):
            p_t = psb.tile([1, GW], F16, tag="p", name="p_t")
            for h in range(NH):
                plog = pslog.tile([1, PGW], F32, tag="plog", name="plog_t")
                for s in range(PGW // 512):
                    for cc in range(CCH):
                        nc.tensor.matmul(
                            plog[:, s * 512:(s + 1) * 512],
                            qks_sb[:, cc:cc + 1],
                            xcache[(cc, g)][:, h * PGW + s * 512:h * PGW + (s + 1) * 512],
                            start=(cc == 0), stop=(cc == CCH - 1),
                        )
                gh = g * NH + h
                nc.scalar.activation(
                    p_t[:, h * PGW:(h + 1) * PGW], plog[:], AF.Exp,
                    accum_out=zcols[:, gh:gh + 1],
                )
            pb = pbp.tile([P, GW], F16, tag="pb", name="pb_t")
            nc.gpsimd.partition_broadcast(pb[:], p_t[:])
            for cc in range(CCH):
                sc = scr.tile([P, GW], F16, tag="scr", name="scr_t")
                nc.vector.scalar_tensor_tensor(
                    sc[:], xcache[(cc, g)][:], 1.0, pb[:],
                    op0=OP.mult, op1=OP.mult,
                    accum_out=pcols[cc][:, g:g + 1],
                )

        for g in range(G):
            emit_group_loads(g)
            if g >= 1:
                emit_group_compute(g - 1)
        emit_group_compute(G - 1)

        # ---- finalize: Z, pooled, gate = (Wv @ pooled)/Z + bv ----
        nc.vector.reduce_sum(z_sb[:], zcols[:], axis=mybir.AxisListType.X)
        zps = pssm.tile([P, 1], F32, tag="pssm", name="pssm_t")
        nc.tensor.matmul(zps[:], ones_sb[:], z_sb[:])
        nc.vector.reciprocal(rz_sb[:], zps[:])
        for cc in range(CCH):
            nc.vector.reduce_sum(
                pooled_sb[:, cc:cc + 1], pcols[cc][:], axis=mybir.AxisListType.X
            )
        for oc in range(CCH):
            pg = pssm.tile([P, 1], F32, tag="pssm", name="pssm_t")
            for cc in range(CCH):
                nc.tensor.matmul(
                    pg[:], wvt_sb[cc][:, oc * P:(oc + 1) * P], pooled_sb[:, cc:cc + 1],
                    start=(cc == 0), stop=(cc == CCH - 1),
                )
            nc.vector.scalar_tensor_tensor(
                gate_sb[:, oc:oc + 1], pg[:], rz_sb[:], bv_sb[:, oc:oc + 1],
                op0=OP.mult, op1=OP.add,
            )

        # ---- write phase: out = x_fp16 * gate, straight from the SBUF cache
        # (no x re-read).  Writes alternate over the two HWDGE rings. ----
        for idx in range(G * CCH):
            g, cc = divmod(idx, CCH)
            o = outp.tile([P, GW], F32, tag="o", name="o_t")
            nc.vector.tensor_scalar_mul(o[:], xcache[(cc, g)][:], gate_sb[:, cc:cc + 1])
            eng = nc.sync if idx % 2 == 0 else nc.scalar
            eng.dma_start(outd[cc * P:(cc + 1) * P, g * GW:(g + 1) * GW], o[:])

    nc.compile()
    return nc


_NC = None


def _get_nc():
    global _NC
    if _NC is None:
        _NC = _build_kernel()
    return _NC


def _make_in_maps(x, context, Wq, bq, Wk, bk, Wv, bv):
    x = np.ascontiguousarray(np.asarray(x, dtype=np.float32))
    wqt = np.ascontiguousarray(np.asarray(Wq, dtype=np.float32).T.astype(np.float16))
    wk = np.ascontiguousarray(np.asarray(Wk, dtype=np.float32).astype(np.float16))
    wvt = np.ascontiguousarray(np.asarray(Wv, dtype=np.float32).T)
    bqc = np.ascontiguousarray(np.asarray(bq, dtype=np.float32).reshape(CCH, P).T)
    bvc = np.ascontiguousarray(np.asarray(bv, dtype=np.float32).reshape(CCH, P).T)
    context = np.asarray(context, dtype=np.float32)
    in_maps = []
    for b in range(NCORES):
        ctxc = np.ascontiguousarray(context[b].reshape(CCH, P).T.astype(np.float16))
        in_maps.append({
            "xb": x[b].reshape(C, HW),
            "ctxc": ctxc,
            "wqt": wqt,
            "wk": wk,
            "wvt": wvt,
            "bqc": bqc,
            "bvc": bvc,
        })
    return in_maps


def run_spmd(x, context, Wq, bq, Wk, bk, Wv, bv, **spmd_kwargs):
    """Run on 8 NeuronCores; returns (output (B,C,H,W) f32, BassKernelResults)."""
    nc = _get_nc()
    in_maps = _make_in_maps(x, context, Wq, bq, Wk, bk, Wv, bv)
    res = run_bass_kernel_spmd(nc, in_maps, list(range(NCORES)), **spmd_kwargs)
    out = np.stack([
        np.asarray(res.results[b]["out"], dtype=np.float32).reshape(C, H, W)
        for b in range(NCORES)
    ])
    return out, res


def kernel(x, context, Wq, bq, Wk, bk, Wv, bv):
    out, _ = run_spmd(x, context, Wq, bq, Wk, bk, Wv, bv)
    return out


# revision 7
# speedup vs baseline: 1.1856x; 1.0596x over previous
"""Trainium2 Bass kernel: ContextCrossAttention (B,C,H,W)=(8,512,128,128).

Math per batch element b (algebraically collapsed from the reference):
  q      = Wq @ ctx_b + bq                          (C,)
  qks    = (q @ Wk) * C**-0.5                       (C,)     # logits = qks . x[:, hw] (+ shift, dropped)
  p[hw]  = exp(logits[hw]);  Z = sum(p)                      # softmax shift-invariance: no max-subtract
  pooled = x_b @ p                                  (C,)
  gate   = (Wv @ pooled) / Z + bv                   (C,)
  out_b  = x_b * gate[:, None]

Sharding: pure data-parallel over batch; core i handles batch element i.

Two DMA-bound phases (~64 MiB mandatory HBM traffic per core):
  read phase:  stream x (f32), convert to an fp16 SBUF cache (scalar/DVE 2x
               modes), logits matmuls read the fp16 cache with the qks
               stationary REPLICATED across all 128 PE columns, so the psum
               logits land on all 128 partitions and exp directly produces
               the broadcast p (no partition_broadcast).  The x.p pooled
               accumulation (STT, 1x-only on DVE) is split DVE/GpSimd.
  write phase: out = x_fp16 * gate straight from the cache (zero re-read),
               staged at [128,1024] over three DMA rings (sync/scalar/gpsimd).
The last 2048 columns are processed as two 1024-wide groups to shorten the
post-last-DMA dependency tail before the write phase can start.
"""

import numpy as np
from contextlib import ExitStack

import concourse.bass as bass
import concourse.bacc as bacc
import concourse.tile as tile
from concourse import mybir
from concourse.bass_utils import run_bass_kernel_spmd

F32 = mybir.dt.float32
F16 = mybir.dt.float16
AF = mybir.ActivationFunctionType
OP = mybir.AluOpType

B, C, D, H, W = 8, 512, 512, 128, 128
HW = H * W                      # 16384
P = 128                         # partitions
CCH = C // P                    # 4 channel chunks
NCORES = 8
SCALE = float(C) ** -0.5

GW = 2048                       # full group width (landing tile size)
# groups of (col0, width): 7 full + shrinking tail groups (shorter post-DMA
# dependency chain before the write phase can start)
GROUPS = [(i * GW, GW) for i in range(7)] + [
    (7 * GW, 1024), (7 * GW + 1024, 512), (7 * GW + 1536, 512)]


def _stt_widths(w):
    # pooled-accumulation unit widths for a group: 1024s plus a 512 residual
    return [1024] * (w // 1024) + ([512] if w % 1024 else [])


N_CHUNKS = sum(w // 512 for _, w in GROUPS)            # 512-wide logits chunks: 32
N_HALVES = sum(len(_stt_widths(w)) for _, w in GROUPS)  # STT units per channel chunk


def _build_kernel():
    nc = bacc.Bacc(
        "TRN2",
        target_bir_lowering=False,
        debug=False,
        enable_asserts=False,
        num_devices=NCORES,
    )

    xd = nc.dram_tensor("xb", [C, HW], F32, kind="ExternalInput")
    ctxd = nc.dram_tensor("ctxc", [P, CCH], F16, kind="ExternalInput")   # ctx[j*128+p] at [p, j]
    wqtd = nc.dram_tensor("wqt", [D, C], F16, kind="ExternalInput")      # Wq.T  (d, o)
    wkd = nc.dram_tensor("wk", [C, C], F16, kind="ExternalInput")        # Wk    (o, c)
    wvtd = nc.dram_tensor("wvt", [C, C], F32, kind="ExternalInput")      # Wv.T  (c, o)
    bqd = nc.dram_tensor("bqc", [P, CCH], F32, kind="ExternalInput")
    bvd = nc.dram_tensor("bvc", [P, CCH], F32, kind="ExternalInput")
    outd = nc.dram_tensor("out", [C, HW], F32, kind="ExternalOutput")

    with tile.TileContext(nc) as tc, ExitStack() as ctx:
        singles = ctx.enter_context(tc.tile_pool(name="singles", bufs=1))
        xt = ctx.enter_context(tc.tile_pool(name="xt", bufs=4))          # f32 landing
        pbp = ctx.enter_context(tc.tile_pool(name="pbp", bufs=2))        # fp16 p (broadcast via psum)
        scr = ctx.enter_context(tc.tile_pool(name="scr", bufs=2))        # fp16 STT product sink
        outp = ctx.enter_context(tc.tile_pool(name="outp", bufs=3))      # f32 out staging
        pslog = ctx.enter_context(tc.tile_pool(name="pslog", bufs=3, space="PSUM"))
        pssm = ctx.enter_context(tc.tile_pool(name="pssm", bufs=2, space="PSUM"))

        # ---- weights / small inputs on the scalar (Act) DMA ring so the
        # sync (SP) ring carries nothing but the x stream ----
        wqt_sb = [None] * CCH
        wk_sb = [None] * CCH
        wvt_sb = [None] * CCH
        ctx_sb = singles.tile([P, CCH], F16, tag="ctx", name="ctx")
        nc.scalar.dma_start(ctx_sb[:], ctxd[:])
        bq_sb = singles.tile([P, CCH], F32, tag="bq", name="bq")
        nc.scalar.dma_start(bq_sb[:], bqd[:])
        for j in range(CCH):
            wqt_sb[j] = singles.tile([P, C], F16, tag=f"wqt{j}", name=f"wqt{j}")
            nc.scalar.dma_start(wqt_sb[j][:], wqtd[j * P:(j + 1) * P, :])
            wk_sb[j] = singles.tile([P, C], F16, tag=f"wk{j}", name=f"wk{j}")
            nc.scalar.dma_start(wk_sb[j][:], wkd[j * P:(j + 1) * P, :])
        bv_sb = singles.tile([P, CCH], F32, tag="bv", name="bv")
        nc.scalar.dma_start(bv_sb[:], bvd[:])
        for j in range(CCH):
            wvt_sb[j] = singles.tile([P, C], F32, tag=f"wvt{j}", name=f"wvt{j}")
            nc.scalar.dma_start(wvt_sb[j][:], wvtd[j * P:(j + 1) * P, :])

        q_sb = singles.tile([P, CCH], F16, tag="q")
        qks_sb = singles.tile([P, CCH], F16, tag="qks")
        qksb = [singles.tile([P, P], F16, tag=f"qksb{cc}", name=f"qksb{cc}") for cc in range(CCH)]
        pooled_sb = singles.tile([P, CCH], F32, tag="pooled")
        gate_sb = singles.tile([P, CCH], F32, tag="gate")
        zacc = singles.tile([P, N_CHUNKS], F32, tag="zacc")
        pcols = [singles.tile([P, N_HALVES], F32, tag=f"pcols{cc}", name=f"pcols{cc}") for cc in range(CCH)]
        zsum_sb = singles.tile([P, 1], F32, tag="zsum")
        rz_sb = singles.tile([P, 1], F32, tag="rz")

        # ---- q = Wq @ ctx + bq  (chunk-major [P, CCH]) ----
        for oc in range(CCH):
            pq = pssm.tile([P, 1], F32, tag="pssm", name="pssm_t")
            for dc in range(CCH):
                nc.tensor.matmul(
                    pq[:], wqt_sb[dc][:, oc * P:(oc + 1) * P], ctx_sb[:, dc:dc + 1],
                    start=(dc == 0), stop=(dc == CCH - 1),
                )
            nc.vector.tensor_add(q_sb[:, oc:oc + 1], pq[:], bq_sb[:, oc:oc + 1])

        # ---- qks = (q @ Wk) * scale; replicate each chunk column across the
        # 128 stationary columns so logits matmuls write all 128 partitions ----
        for cc in range(CCH):
            pqk = pssm.tile([P, 1], F32, tag="pssm", name="pssm_t")
            for oc in range(CCH):
                nc.tensor.matmul(
                    pqk[:], wk_sb[oc][:, cc * P:(cc + 1) * P], q_sb[:, oc:oc + 1],
                    start=(oc == 0), stop=(oc == CCH - 1),
                )
            nc.scalar.mul(qks_sb[:, cc:cc + 1], pqk[:], SCALE)
            nc.vector.tensor_copy(qksb[cc][:], qks_sb[:, cc:cc + 1].broadcast_to([P, P]))

        # ---- read phase: stream x in, convert to the fp16 cache, fused
        # logits -> exp -> pooled partials.  Converts for group i+1 are
        # emitted BEFORE exp/STT of group i (per-engine FIFO software
        # pipelining) so the landing pool recycles without the x DMA stream
        # stalling behind the exp/STT chain. ----
        xcache = {}
        chunk_base = [0]
        half_base = [0]
        for _, w in GROUPS:
            chunk_base.append(chunk_base[-1] + w // 512)
            half_base.append(half_base[-1] + len(_stt_widths(w)))

        def emit_group_loads(gi):
            col0, w = GROUPS[gi]
            for cc in range(CCH):
                t = xt.tile([P, GW], F32, tag="x", name="x_t")
                nc.sync.dma_start(t[:, :w], xd[cc * P:(cc + 1) * P, col0:col0 + w])
                xc = singles.tile([P, w], F16, tag=f"xc{gi}_{cc}", name=f"xc{gi}_{cc}")
                if cc % 2 == 0:
                    nc.scalar.copy(xc[:], t[:, :w])
                else:
                    nc.vector.tensor_copy(xc[:], t[:, :w])
                xcache[(cc, gi)] = xc

        def emit_group_compute(gi):
            col0, w = GROUPS[gi]
            pb = pbp.tile([P, GW], F16, tag="pb", name="pb_t")
            hoff = 0
            for h, hw_ in enumerate(_stt_widths(w)):
                for s2 in range(hw_ // 512):
                    s = (hoff // 512) + s2
                    plog = pslog.tile([P, 512], F32, tag="plog", name="plog_t")
                    for cc in range(CCH):
                        nc.tensor.matmul(
                            plog[:], qksb[cc][:],
                            xcache[(cc, gi)][:, s * 512:(s + 1) * 512],
                            start=(cc == 0), stop=(cc == CCH - 1),
                        )
                    ch = chunk_base[gi] + s
                    nc.scalar.activation(
                        pb[:, s * 512:(s + 1) * 512], plog[:], AF.Exp,
                        accum_out=zacc[:, ch:ch + 1],
                    )
                gh = half_base[gi] + h
                for cc in range(CCH):
                    sc = scr.tile([P, 1024], F16, tag="scr", name="scr_t")
                    nc.vector.scalar_tensor_tensor(
                        sc[:, :hw_], xcache[(cc, gi)][:, hoff:hoff + hw_], 1.0,
                        pb[:, hoff:hoff + hw_],
                        op0=OP.mult, op1=OP.mult,
                        accum_out=pcols[cc][:, gh:gh + 1],
                    )
                hoff += hw_

        NG = len(GROUPS)
        for gi in range(NG):
            emit_group_loads(gi)
            if gi >= 1:
                emit_group_compute(gi - 1)
        emit_group_compute(NG - 1)

        # ---- finalize: Z, pooled, gate = (Wv @ pooled)/Z + bv.
        # Every partition's zacc row sums the same 16384 p values, so the
        # reduction directly yields Z on all partitions (no broadcast). ----
        nc.vector.reduce_sum(zsum_sb[:], zacc[:], axis=mybir.AxisListType.X)
        nc.vector.reciprocal(rz_sb[:], zsum_sb[:])
        for cc in range(CCH):
            nc.vector.reduce_sum(
                pooled_sb[:, cc:cc + 1], pcols[cc][:], axis=mybir.AxisListType.X
            )
        for oc in range(CCH):
            pg = pssm.tile([P, 1], F32, tag="pssm", name="pssm_t")
            for cc in range(CCH):
                nc.tensor.matmul(
                    pg[:], wvt_sb[cc][:, oc * P:(oc + 1) * P], pooled_sb[:, cc:cc + 1],
                    start=(cc == 0), stop=(cc == CCH - 1),
                )
            nc.vector.scalar_tensor_tensor(
                gate_sb[:, oc:oc + 1], pg[:], rz_sb[:], bv_sb[:, oc:oc + 1],
                op0=OP.mult, op1=OP.add,
            )

        # ---- write phase: out = x_fp16 * gate straight from the SBUF cache
        # (no x re-read), staged at [128,1024] across three DMA rings ----
        wr_engines = [nc.sync, nc.scalar, nc.gpsimd]
        widx = 0
        for gi in range(NG):
            col0, w = GROUPS[gi]
            for cc in range(CCH):
                hoff = 0
                for hw_ in _stt_widths(w):
                    o = outp.tile([P, 1024], F32, tag="o", name="o_t")
                    nc.vector.tensor_scalar_mul(
                        o[:, :hw_], xcache[(cc, gi)][:, hoff:hoff + hw_],
                        gate_sb[:, cc:cc + 1],
                    )
                    eng = wr_engines[widx % 3]
                    widx += 1
                    eng.dma_start(
                        outd[cc * P:(cc + 1) * P, col0 + hoff:col0 + hoff + hw_],
                        o[:, :hw_],
                    )
                    hoff += hw_

    nc.compile()
    return nc


_NC = None


def _get_nc():
    global _NC
    if _NC is None:
        _NC = _build_kernel()
    return _NC


def _make_in_maps(x, context, Wq, bq, Wk, bk, Wv, bv):
    x = np.ascontiguousarray(np.asarray(x, dtype=np.float32))
    wqt = np.ascontiguousarray(np.asarray(Wq, dtype=np.float32).T.astype(np.float16))
    wk = np.ascontiguousarray(np.asarray(Wk, dtype=np.float32).astype(np.float16))
    wvt = np.ascontiguousarray(np.asarray(Wv, dtype=np.float32).T)
    bqc = np.ascontiguousarray(np.asarray(bq, dtype=np.float32).reshape(CCH, P).T)
    bvc = np.ascontiguousarray(np.asarray(bv, dtype=np.float32).reshape(CCH, P).T)
    context = np.asarray(context, dtype=np.float32)
    in_maps = []
    for b in range(NCORES):
        ctxc = np.ascontiguousarray(context[b].reshape(CCH, P).T.astype(np.float16))
        in_maps.append({
            "xb": x[b].reshape(C, HW),
            "ctxc": ctxc,
            "wqt": wqt,
            "wk": wk,
            "wvt": wvt,
            "bqc": bqc,
            "bvc": bvc,
        })
    return in_maps


def run_spmd(x, context, Wq, bq, Wk, bk, Wv, bv, **spmd_kwargs):
    """Run on 8 NeuronCores; returns (output (B,C,H,W) f32, BassKernelResults)."""
    nc = _get_nc()
    in_maps = _make_in_maps(x, context, Wq, bq, Wk, bk, Wv, bv)
    res = run_bass_kernel_spmd(nc, in_maps, list(range(NCORES)), **spmd_kwargs)
    out = np.stack([
        np.asarray(res.results[b]["out"], dtype=np.float32).reshape(C, H, W)
        for b in range(NCORES)
    ])
    return out, res


def kernel(x, context, Wq, bq, Wk, bk, Wv, bv):
    out, _ = run_spmd(x, context, Wq, bq, Wk, bk, Wv, bv)
    return out
